# revision 35
# baseline (speedup 1.0000x reference)
"""Trainium2 Bass kernel for nn_BigAttention (weight-norm MLP + softmax-over-k).

Math (per the reference):
    W1e = g1 * W1 / ||W1||_F          [1024, 3072]
    W2e = g2 * W2 / ||W2||_F          [1, 1024]
    hv  = v @ W1e[:, :2048].T         [B,K,N,1024]
    hq  = q @ W1e[:, 2048:].T         [B,K,1024]
    joint  = relu(hv + hq + b1)
    logits = joint @ W2e.T  (+ b2, which cancels in the softmax over k)
    out = softmax(logits, axis=K)     [B,K,N,1]

Sharding: data-parallel over batch, 8 batches per core; weights replicated.

Precision: the big contractions run in fp8e4m3 with the PE's DoubleRow mode
(2 fp8 contraction chunks per matmul -> ~2x the bf16 matmul rate).  W1e's
entries are ~5.6e-4 rms (weight_norm divides by ||W1||_F ~ 35), far below
fp8's normal range, so the host scales W1e by S=1024 before quantizing and
folds 1/S into the fp32 W2 epilogue (relu commutes with the positive scale).
Measured end-to-end max rel err ~3.5e-3 (tolerance 2e-2).

Per-core device program (rows r = (b_local, k, n) flattened, R = 8*12*36 = 3456):
  - ~3.6us of junk DoubleRow matmuls first: no DMA deps, so the PE HAM
    clock-gate reaches 8/8 while the startup uploads stream.
  - hq[96, 1024] via fp8 DoubleRow (q^T/W1q^T pairs, chunk-paced 4-way DMA),
    b1*S folded in on the DVE eviction; written once to DRAM.
  - main: per 128-row tile, PSUM[row, hidden 1024] accumulates 8 DoubleRow
    fp8 matmuls (v^T pair stationary, W1v^T pair moving) per 512-half -- a
    pure uniform-DR stream, 16 matmuls/tile at ~518 PE cycles each.  The
    hq add does NOT touch the PE: per tile, 4-5 tiny broadcast-DMAs gather
    the per-row hq vectors from DRAM (partition-replicating a 2KB row), and
    the DVE adds them during eviction.
  - epilogue per tile: DVE pass 1 writes joint = psum + hq_rows (bf16);
    DVE pass 2 computes (joint max 0) * w2_broadcast with accum_out =
    per-row sum = the logit.
  - chunk 0 runs its three tiles cp-major (each W1v upload group is
    consumed the moment it lands) with tile-major cp6/7 tails so each
    tile's eviction overlaps the next tile's matmuls; later chunks are
    tile-major with the next v chunk's DMA issued mid-chunk (never
    head-blocking a ring).  The very last tile streams its two 512-column
    halves back-to-back so half the eviction hides under the final
    matmuls.
  - softmax over k: logits collect in [128 p, tile] tiles; 4 DVE 32x32
    block transposes build a true [tile, 128] transpose; one DMA round
    trip through linear DRAM re-groups rows into [96 (b,k), 36 n].
    bk 0:64 flushes at tile 17 and bk 64:92 at tile 25 (both hidden under
    the main loop); only the last 4 (b,k) rows ride the critical tail via
    3 direct SBUF->SBUF run copies into a 64-base staging tile (legal
    matmul base).  exp on ACT; per-(b,n) sums and the 1/sum broadcast are
    tiny f32r one-hot matmuls; final scale on DVE; two parallel DMAs
    write the [8,12,36,1] output slice.

All heavy inputs are host-repacked "partition-major" so every big DMA is
128 contiguous runs (one per partition).  Uploads are split across the
sync- and scalar-engine HWDGE rings in consumption order; the aggregate
~400GB/s HBM bandwidth is the startup binding constraint.
"""

import ml_dtypes
import numpy as np

import concourse.bacc as bacc
import concourse.mybir as mybir
import concourse.tile as tile
from concourse.bass_utils import run_bass_kernel_spmd

F32 = mybir.dt.float32
F32R = mybir.dt.float32r
NCORES = 8
B, K, N = 64, 12, 36
VD, QD, HID = 2048, 1024, 1024
BL = B // NCORES              # local batches per core
R = BL * K * N                # 3456 rows per core
BK = BL * K                   # 96 (b,k) groups per core
CC = VD // 128                # 16 contraction chunks over v-dim
QC = QD // 128                # 8 contraction chunks over q-dim
RC = 384                      # rows per DMA chunk (9 chunks)
NCH = R // RC
RT = 128                      # rows per PSUM tile
NT = RC // RT
NRT = R // RT                 # 27 row tiles
VSPLIT = 16                   # v-chunk DMA granularity (cc chunks per DMA)
SCALE = 1024.0                # host-side W1 scale so fp8 sees O(1) weights

_NC_CACHE = None

F8 = mybir.dt.float8e4
BF16 = mybir.dt.bfloat16
DR = mybir.MatmulPerfMode.DoubleRow


def _build_nc():
    nc = bacc.Bacc("TRN2", target_bir_lowering=False, debug=False,
                   num_devices=NCORES)

    def mm(out, lhsT, rhs, **kw):
        nc.tensor.matmul(out, lhsT, rhs, **kw)

    w1vt = nc.dram_tensor("w1vt", [128, CC, HID], F8, kind="ExternalInput").ap()
    # qt and W1q^T packed along the free dim: [:, cq, 0:96]=q^T, [:, cq, 96:1120]=W1q^T
    qtwq = nc.dram_tensor("qtwq", [128, QC, BK + HID], F8, kind="ExternalInput").ap()
    # bf16 constants: [:, 0:1024]=w2/S bcast, [0:96, 1024:2048]=b1*S replicated
    packb = nc.dram_tensor("packb", [128, HID + HID], BF16, kind="ExternalInput").ap()
    # f32r softmax selectors: [0:96, 0:8]=selb, [0:8, 8:104]=selb^T,
    # [64:68, 104:112]=selb rows 92:96 (the base-64 remainder copy),
    # [0:8, 112:180]=selb^T cols 92:96 remapped to out rows 64:68
    packf = nc.dram_tensor("packf", [128, BL + BK + BL + 68], F32R, kind="ExternalInput").ap()
    # v is split: the first two chunks ride with the weights at the front of
    # the upload order; the bulk uploads last, hidden under early compute.
    vth = nc.dram_tensor("vth", [2, 128, CC, RC], F8, kind="ExternalInput").ap()
    vtr = nc.dram_tensor("vtr", [NCH - 2, 128, CC, RC], F8, kind="ExternalInput").ap()
    out = nc.dram_tensor("out", [BL, K, N, 1], F32, kind="ExternalOutput").ap()

    MAX = mybir.AluOpType.max
    MULT = mybir.AluOpType.mult
    BYPASS = mybir.AluOpType.bypass
    ADD = mybir.AluOpType.add

    with tile.TileContext(nc) as tc:
        with tc.tile_pool(name="const", bufs=1) as cpool, \
             tc.tile_pool(name="wv", bufs=1) as wvpool, \
             tc.tile_pool(name="vtp", bufs=2) as vtpool, \
             tc.tile_pool(name="work", bufs=3) as work, \
             tc.tile_pool(name="small", bufs=1) as small, \
             tc.tile_pool(name="dram", bufs=1, space="DRAM") as dpool, \
             tc.tile_pool(name="psum", bufs=4, space="PSUM") as pspool:

            # ---- startup uploads, issue order == consumption order.
            qt4 = []
            for j in range(QC // 2):
                t = cpool.tile([128, 2, BK + HID], F8, name=f"qt{j}")
                nc.sync.dma_start(out=t, in_=qtwq[:, 2 * j:2 * j + 2, :])
                qt4.append(t)

            def vt_chunk_tiles(ch):
                src_ap = vth[ch] if ch < 2 else vtr[ch - 2]
                tiles = []
                eng = nc.scalar if ch == 0 else nc.sync
                for j in range(CC // VSPLIT):
                    t = vtpool.tile([128, VSPLIT, RC], F8, tag=f"vt{j}")
                    eng.dma_start(
                        out=t, in_=src_ap[:, j * VSPLIT:(j + 1) * VSPLIT, :])
                    tiles.append(t)
                return tiles

            vt_cur = vt_chunk_tiles(0)

            packb_s = cpool.tile([128, HID + HID], BF16)
            w2b_s = packb_s[:, 0:HID]
            b1b_s = packb_s[0:BK, HID:HID + HID]
            # b1 half first: it gates the hq eviction
            nc.sync.dma_start(out=packb_s[:, HID:HID + HID],
                              in_=packb[:, HID:HID + HID])
            nc.sync.dma_start(out=packb_s[:, 0:HID], in_=packb[:, 0:HID])

            WG = 4  # wv group size (cc chunks per DMA)
            wv_g = []
            for j in range(CC // WG):
                t = wvpool.tile([128, WG, HID], F8, tag=f"wvg{j}")
                eng = nc.scalar if j < 3 else nc.sync
                eng.dma_start(out=t, in_=w1vt[:, j * WG:(j + 1) * WG, :])
                wv_g.append(t)

            packf_s = cpool.tile([128, BL + BK + BL + 68], F32R)
            nc.sync.dma_start(out=packf_s, in_=packf)

            vt_next = vt_chunk_tiles(1)

            selb_s = packf_s[0:BK, 0:BL]
            selbt_s = packf_s[0:BL, BL:BL + BK]

            # per-row logits, laid out [p, rt] with row = rt*128 + p, split
            # into two tiles so the first half's softmax hides under the
            # main loop. 32 columns (StreamTranspose needs 32x32 blocks).
            NRT_A = 18   # 18*128 rows = 64 (b,k) groups — a 32-aligned bk split
            ls_a = cpool.tile([128, 32], F32)
            nc.vector.memset(ls_a, 0.0)
            ls_b = cpool.tile([128, 32], F32)
            nc.vector.memset(ls_b, 0.0)

            lg = dpool.tile([R], F32)
            lg2 = lg.rearrange("(t p) -> t p", t=NRT, p=128)

            hq_s = cpool.tile([BK, HID], BF16)
            s96 = small.tile([BK, N], F32)
            e96 = small.tile([BK, N], F32R)
            s96c = small.tile([68, N], F32, name="s96c")
            e96c = small.tile([68, N], F32R, name="e96c")
            sums_ps = pspool.tile([BL, N], F32, tag="sm", bufs=2)
            lst2a = cpool.tile([32, 128], F32, name="lst2a")
            lst2b = cpool.tile([32, 128], F32, name="lst2b")
            lgbk = lg.rearrange("(bk n) -> bk n", n=N)

            def build_lst2(ls, ls_t2):
                # ls[p, c] -> ls_t2[c, p] via 4 32x32 block transposes
                for i in range(4):
                    nc.vector.transpose(
                        ls_t2[0:32, 32 * i:32 * i + 32],
                        ls[32 * i:32 * i + 32, 0:32])

            PV = VSPLIT // 2   # DoubleRow cc-pairs per v DMA tile
            PW = WG // 2       # DoubleRow cc-pairs per wv group

            def emit_vmms(t, ps):
                for cp in range(CC // 2):
                    lhsT = vt_cur[cp // PV][:, (cp % PV) * 2:(cp % PV) * 2 + 2,
                                            t * RT:(t + 1) * RT]
                    wvc = wv_g[cp // PW][:, (cp % PW) * 2:(cp % PW) * 2 + 2, :]
                    mm(ps[:, 0:512], lhsT, wvc[:, :, 0:512],
                       start=(cp == 0), stop=(cp == CC // 2 - 1), perf_mode=DR)
                    mm(ps[:, 512:1024], lhsT, wvc[:, :, 512:1024],
                       start=(cp == 0), stop=(cp == CC // 2 - 1), perf_mode=DR)

            def emit_gather(rt):
                # broadcast-DMA the per-row hq vectors for this tile's rows
                # out of DRAM: one small DMA per (b,k) group intersecting the
                # tile, each replicating a single 4KB row across partitions.
                hqr = work.tile([128, HID], BF16, tag="hqr", bufs=3)
                g0 = (rt * RT) // N
                g1 = (rt * RT + RT - 1) // N
                eng = nc.sync if rt % 2 == 0 else nc.scalar
                for g in range(g0, g1 + 1):
                    a = max(0, N * g - rt * RT)
                    b = min(RT, N * (g + 1) - rt * RT)
                    eng.dma_start(
                        out=hqr[a:b, :],
                        in_=hq_dram[g:g + 1, :].to_broadcast((b - a, HID)))
                return hqr

            def emit_closer(rt, ps, hqr):
                joint = work.tile([128, HID], BF16, tag="joint")
                nc.vector.scalar_tensor_tensor(
                    out=joint, in0=ps, scalar=0.0, in1=hqr,
                    op0=BYPASS, op1=ADD)
                relu_w2 = work.tile([128, HID], BF16, tag="relu_w2")
                ls, col = (ls_a, rt) if rt < NRT_A else (ls_b, rt - NRT_A)
                nc.vector.scalar_tensor_tensor(
                    out=relu_w2, in0=joint, scalar=0.0, in1=w2b_s,
                    op0=MAX, op1=MULT,
                    accum_out=ls[:, col:col + 1])
                if rt == NRT_A - 1:
                    # flush + start the softmax head for bk rows 0:64 while
                    # the main loop still runs
                    build_lst2(ls_a, lst2a)
                    nc.sync.dma_start(out=lg2[0:NRT_A, :],
                                      in_=lst2a[0:NRT_A, :])
                    nc.sync.dma_start(out=s96[0:64, :], in_=lgbk[0:64, :])
                    nc.scalar.activation(e96[0:64, :], s96[0:64, :],
                                         mybir.ActivationFunctionType.Exp)
                    mm(sums_ps, selb_s[0:64, :], e96[0:64, :],
                       start=True, stop=False)
                if rt == NRT - 2:
                    # stage bk 64:92 (fully contained in tiles 18..25) so
                    # only the 4-row remainder rides the critical tail
                    build_lst2(ls_b, lst2b)
                    nc.sync.dma_start(out=lg2[NRT_A:NRT - 1, :],
                                      in_=lst2b[0:NRT - 1 - NRT_A, :])
                    nc.sync.dma_start(out=s96[64:92, :], in_=lgbk[64:92, :])
                    nc.scalar.activation(e96[64:92, :], s96[64:92, :],
                                         mybir.ActivationFunctionType.Exp)
                    mm(sums_ps, selb_s[64:92, :], e96[64:92, :],
                       start=False, stop=False)

            # ---- PE warmup: ~3.6us of junk DoubleRow matmuls with no DMA
            # dependencies, so the HAM clock-gate reaches 8/8 while the
            # startup uploads are still streaming.
            junk = work.tile([128, 1280], F8, tag="junk", bufs=1)
            nc.gpsimd.memset(junk, 0.0)
            junk_ps = pspool.tile([128, 512], F32, tag="sm", bufs=2,
                                  name="junk_ps")
            for _ in range(14):
                mm(junk_ps, junk.rearrange("p (two m) -> p two m", two=2)[:, :, 0:128],
                   junk.rearrange("p (two m) -> p two m", two=2)[:, :, 128:640],
                   start=True, stop=True, perf_mode=DR)

            # ---- hq[bk, h] = q @ W1q^T (+ b1*S on eviction), via fp8
            # DoubleRow; first in PE program order so the PE starts as soon
            # as qtwq (the first upload) lands.
            hq_ps = [pspool.tile([BK, 512], F32, tag="sm", bufs=2,
                                 name=f"hq_ps{i}") for i in range(2)]
            for cp in range(QC // 2):
                src_t = qt4[cp]
                for half in range(2):
                    mm(hq_ps[half],
                       src_t[:, :, 0:BK],
                       src_t[:, :, BK + half * 512:BK + (half + 1) * 512],
                       start=(cp == 0), stop=(cp == QC // 2 - 1),
                       perf_mode=DR)
            for half in range(2):
                hs = slice(half * 512, (half + 1) * 512)
                # psum -> SBUF with the b1*S row added
                nc.vector.scalar_tensor_tensor(
                    out=hq_s[:, hs], in0=hq_ps[half], scalar=0.0,
                    in1=b1b_s[:, hs],
                    op0=BYPASS, op1=ADD)
            hq_dram = dpool.tile([BK, HID], BF16)
            nc.scalar.dma_start(out=hq_dram, in_=hq_s)

            # short filler so the PE never idles across a HAM window while
            # the first W1v group lands
            junk_ps2 = pspool.tile([128, 512], F32, tag="sm", bufs=2,
                                   name="junk_ps2")
            for _ in range(9):
                mm(junk_ps2,
                   junk.rearrange("p (two m) -> p two m", two=2)[:, :, 0:128],
                   junk.rearrange("p (two m) -> p two m", two=2)[:, :, 128:640],
                   start=True, stop=True, perf_mode=DR)

            # ---- chunk 0: v-matmuls for tiles 0..2 first; the closers are
            # deferred until the hq/oneh constants are certainly ready.
            ps0 = [pspool.tile([128, HID], F32, tag="ps", bufs=3,
                               name=f"ps0_{t}") for t in range(NT)]
            hqr0 = [emit_gather(t) for t in range(NT)]
            # cp-major: consume each wv group across all 3 tiles as soon as
            # its upload lands, so the PE never waits for the full W1v set.
            for cp in range(CC // 2 - 2):
                wvc = wv_g[cp // PW][:, (cp % PW) * 2:(cp % PW) * 2 + 2, :]
                for t in range(NT):
                    lhsT = vt_cur[cp // PV][:, (cp % PV) * 2:(cp % PV) * 2 + 2,
                                            t * RT:(t + 1) * RT]
                    mm(ps0[t][:, 0:512], lhsT, wvc[:, :, 0:512],
                       start=(cp == 0), stop=False, perf_mode=DR)
                    mm(ps0[t][:, 512:1024], lhsT, wvc[:, :, 512:1024],
                       start=(cp == 0), stop=False, perf_mode=DR)
                if cp % 2 == 1:
                    # keep the PE HAM-warm across each wv upload boundary
                    for _ in range(2):
                        mm(junk_ps2,
                           junk.rearrange("p (two m) -> p two m", two=2)[:, :, 0:128],
                           junk.rearrange("p (two m) -> p two m", two=2)[:, :, 128:640],
                           start=True, stop=True, perf_mode=DR)
            # tile-major tails: each tile's eviction starts while the next
            # tile's last matmuls still stream
            for t in range(NT):
                for cp in (CC // 2 - 2, CC // 2 - 1):
                    wvc = wv_g[cp // PW][:, (cp % PW) * 2:(cp % PW) * 2 + 2, :]
                    lhsT = vt_cur[cp // PV][:, (cp % PV) * 2:(cp % PV) * 2 + 2,
                                            t * RT:(t + 1) * RT]
                    mm(ps0[t][:, 0:512], lhsT, wvc[:, :, 0:512],
                       start=False, stop=(cp == CC // 2 - 1), perf_mode=DR)
                    mm(ps0[t][:, 512:1024], lhsT, wvc[:, :, 512:1024],
                       start=False, stop=(cp == CC // 2 - 1), perf_mode=DR)
                emit_closer(t, ps0[t], hqr0[t])
            vt_cur = vt_next

            # ---- chunks 1..8
            for ch in range(1, NCH):
                for t in range(NT):
                    rt = ch * NT + t
                    hqr = emit_gather(rt)
                    if t == 0 and ch + 1 < NCH:
                        vt_next = vt_chunk_tiles(ch + 1)
                    ps = pspool.tile([128, HID], F32, tag="ps", bufs=3)
                    if rt == NRT - 1:
                        # half-split: evict half 0 while half 1 still streams
                        lsh = work.tile([128, 2], F32, tag="acc2")
                        jointx = work.tile([128, HID], BF16, tag="joint")
                        rwx = work.tile([128, HID], BF16, tag="relu_w2")
                        for half in range(2):
                            hs = slice(half * 512, (half + 1) * 512)
                            for cp in range(CC // 2):
                                lhsT = vt_cur[cp // PV][
                                    :, (cp % PV) * 2:(cp % PV) * 2 + 2,
                                    t * RT:(t + 1) * RT]
                                wvc = wv_g[cp // PW][
                                    :, (cp % PW) * 2:(cp % PW) * 2 + 2, :]
                                mm(ps[:, hs], lhsT, wvc[:, :, hs],
                                   start=(cp == 0),
                                   stop=(cp == CC // 2 - 1), perf_mode=DR)
                            nc.vector.scalar_tensor_tensor(
                                out=jointx[:, hs], in0=ps[:, hs], scalar=0.0,
                                in1=hqr[:, hs], op0=BYPASS, op1=ADD)
                            nc.vector.scalar_tensor_tensor(
                                out=rwx[:, hs], in0=jointx[:, hs], scalar=0.0,
                                in1=w2b_s[:, hs], op0=MAX, op1=MULT,
                                accum_out=lsh[:, half:half + 1])
                        nc.vector.scalar_tensor_tensor(
                            out=ls_b[:, NRT - 1 - NRT_A:NRT - NRT_A],
                            in0=lsh[:, 0:1], scalar=0.0, in1=lsh[:, 1:2],
                            op0=BYPASS, op1=ADD)
                    else:
                        emit_vmms(t, ps)
                        emit_closer(rt, ps, hqr)
                vt_cur = vt_next

            # ---- flush the last tile's logits, finish the softmax.
            # The 4 remaining (b,k) rows (bk 92:96 = r 3312:3456) live in
            # lst2b partitions 7:9; copy their runs straight SBUF->SBUF
            # (1-descriptor DMAs), once into s96[92:96] for the final scale
            # and once into s96c[64:68] (a legal 64-base copy for the
            # partial-sums matmul).
            nc.sync.dma_start(out=s96c[64:65, 0:16],
                              in_=ls_b[112:128, 7:8])
            nc.scalar.dma_start(out=s96c[64:65, 16:36],
                                in_=ls_b[0:20, 8:9])
            nc.sync.dma_start(out=s96c[65:68, :], in_=ls_b[20:128, 8:9])
            nc.scalar.activation(e96c[64:68, :], s96c[64:68, :],
                                 mybir.ActivationFunctionType.Exp)
            mm(sums_ps, packf_s[64:68, BL + BK:BL + BK + BL], e96c[64:68, :],
               start=False, stop=True)
            rcp = small.tile([BL, N], F32R)
            with nc.allow_low_precision(reason="f32r is full fp32 bits"):
                nc.vector.reciprocal(rcp, sums_ps)
            rexp_ps = pspool.tile([BK, N], F32, tag="sm", bufs=2)
            mm(rexp_ps, selbt_s, rcp, start=True, stop=True)
            # the 4 remainder rows ride a 64-base copy: col-tiled broadcast
            # matmul + their own scale/DMA
            rexpc_ps = pspool.tile([68, N], F32, tag="sm", bufs=2,
                                   name="rexpc_ps")
            mm(rexpc_ps, packf_s[0:BL, BL + BK + BL:BL + BK + BL + 68], rcp,
               start=True, stop=True)
            w96 = small.tile([BK, N], F32)
            nc.vector.scalar_tensor_tensor(
                out=w96[0:92, :], in0=e96[0:92, :], scalar=0.0,
                in1=rexp_ps[0:92, :], op0=BYPASS, op1=MULT)
            w96x = small.tile([68, N], F32, name="w96x")
            nc.vector.scalar_tensor_tensor(
                out=w96x[64:68, :], in0=e96c[64:68, :], scalar=0.0,
                in1=rexpc_ps[64:68, :], op0=BYPASS, op1=MULT)
            outbk = out.rearrange("b k n o -> (b k) (n o)")
            nc.sync.dma_start(out=outbk[0:92, :], in_=w96[0:92, :])
            nc.scalar.dma_start(out=outbk[92:96, :], in_=w96x[64:68, :])

    nc.compile()
    return nc


def _get_nc():
    global _NC_CACHE
    if _NC_CACHE is None:
        _NC_CACHE = _build_nc()
    return _NC_CACHE


def _prepare_in_maps(inputs):
    v = np.asarray(inputs["v"], dtype=np.float32)
    q = np.asarray(inputs["q"], dtype=np.float32)
    W1 = np.asarray(inputs["W1"], dtype=np.float32)
    g1 = np.float64(np.asarray(inputs["g1"]))
    b1 = np.asarray(inputs["b1"], dtype=np.float32)
    W2 = np.asarray(inputs["W2"], dtype=np.float32)
    g2 = np.float64(np.asarray(inputs["g2"]))
    # b2 is a scalar added to every logit -> cancels in softmax over k.

    W1e = ((g1 / np.linalg.norm(W1.astype(np.float64))) * W1).astype(np.float32)
    W2e = ((g2 / np.linalg.norm(W2.astype(np.float64))) * W2).astype(np.float32)

    BF = ml_dtypes.bfloat16
    FP8 = ml_dtypes.float8_e4m3
    W1s = (W1e * SCALE).astype(np.float32)
    # partition-major repacks: [..., 128 p, chunk, inner]
    w1vt = np.ascontiguousarray(                       # [128, 16, 1024]
        W1s[:, :VD].T.reshape(CC, 128, HID).transpose(1, 0, 2)).astype(FP8)
    w1qt = W1s[:, VD:].T.reshape(QC, 128, HID).transpose(1, 0, 2)  # [128, 8, 1024]
    selb = (np.arange(BL)[None, :] == (np.arange(BK) // K)[:, None]).astype(np.float32)


    packb = np.zeros((128, HID + HID), dtype=BF)
    packb[:, 0:HID] = (W2e.reshape(1, HID) / SCALE).astype(BF)
    packb[0:BK, HID:HID + HID] = (b1.reshape(1, HID) * SCALE).astype(BF)
    packf = np.zeros((128, BL + BK + BL + 68), dtype=np.float32)
    packf[0:BK, 0:BL] = selb
    packf[0:BL, BL:BL + BK] = selb.T
    packf[64:68, BL + BK:BL + BK + BL] = selb[92:96]
    packf[0:BL, BL + BK + BL + 64:BL + BK + BL + 68] = selb.T[:, 92:96]

    shared = dict(w1vt=w1vt, packb=packb, packf=packf)
    in_maps = []
    for c in range(NCORES):
        vl = v[c * BL:(c + 1) * BL].reshape(R, VD)
        # vt[ch, p, cc, r_in_chunk] = v[ch*RC + r, cc*128 + p]
        vt4 = np.ascontiguousarray(
            vl.T.reshape(CC, 128, NCH, RC).transpose(2, 1, 0, 3)).astype(FP8)
        ql = q[c * BL:(c + 1) * BL].reshape(BK, QD)
        qt3 = ql.T.reshape(QC, 128, BK).transpose(1, 0, 2)   # [128, 8, 96]
        qtwq = np.concatenate([qt3, w1qt], axis=2).astype(FP8)  # [128, 8, 1120]
        in_maps.append(dict(vth=np.ascontiguousarray(vt4[:2]),
                            vtr=np.ascontiguousarray(vt4[2:]),
                            qtwq=np.ascontiguousarray(qtwq), **shared))
    return in_maps


def kernel(**inputs) -> np.ndarray:
    in_maps = _prepare_in_maps(inputs)
    nc = _get_nc()
    res = run_bass_kernel_spmd(nc, in_maps, list(range(NCORES)))
    outs = [res.results[c]["out"].reshape(BL, K, N, 1) for c in range(NCORES)]
    return np.concatenate(outs, axis=0)


# revision 36
# speedup vs baseline: 1.0327x; 1.0327x over previous
"""Trainium2 Bass kernel for nn_BigAttention (weight-norm MLP + softmax-over-k).

Math (per the reference):
    W1e = g1 * W1 / ||W1||_F          [1024, 3072]
    W2e = g2 * W2 / ||W2||_F          [1, 1024]
    hv  = v @ W1e[:, :2048].T         [B,K,N,1024]
    hq  = q @ W1e[:, 2048:].T         [B,K,1024]
    joint  = relu(hv + hq + b1)
    logits = joint @ W2e.T  (+ b2, which cancels in the softmax over k)
    out = softmax(logits, axis=K)     [B,K,N,1]

Sharding: data-parallel over batch, 8 batches per core; weights replicated.

Precision: the big contractions run in fp8e4m3 with the PE's DoubleRow mode
(2 fp8 contraction chunks per matmul -> ~2x the bf16 matmul rate).  W1e's
entries are ~5.6e-4 rms (weight_norm divides by ||W1||_F ~ 35), far below
fp8's normal range, so the host scales W1e by S=1024 before quantizing and
folds 1/S into the fp32 W2 epilogue (relu commutes with the positive scale).
Measured end-to-end max rel err ~3.5e-3 (tolerance 2e-2).

Per-core device program (rows r = (b_local, k, n) flattened, R = 8*12*36 = 3456):
  - ~3.6us of junk DoubleRow matmuls first: no DMA deps, so the PE HAM
    clock-gate reaches 8/8 while the startup uploads stream.
  - hq[96, 1024] via fp8 DoubleRow (q^T/W1q^T pairs, chunk-paced 4-way DMA),
    b1*S folded in on the DVE eviction; written once to DRAM.
  - main: per 128-row tile, PSUM[row, hidden 1024] accumulates 8 DoubleRow
    fp8 matmuls (v^T pair stationary, W1v^T pair moving) per 512-half -- a
    pure uniform-DR stream, 16 matmuls/tile at ~518 PE cycles each.  The
    hq add does NOT touch the PE: per tile, 4-5 tiny broadcast-DMAs gather
    the per-row hq vectors from DRAM (partition-replicating a 2KB row), and
    the DVE adds them during eviction.
  - epilogue per tile: DVE pass 1 writes joint = psum + hq_rows (bf16);
    DVE pass 2 computes (joint max 0) * w2_broadcast with accum_out =
    per-row sum = the logit.
  - chunk 0 runs its three tiles cp-major so each W1v upload group is
    consumed the moment it lands; later chunks are tile-major with the
    next v chunk's DMA issued mid-chunk (never head-blocking a ring).
  - softmax over k: logits collect in [128 p, tile] tiles; 4 DVE 32x32
    block transposes build a true [tile, 128] transpose; one DMA round
    trip through linear DRAM re-groups rows into [96 (b,k), 36 n].
    bk 0:64 flushes at tile 17 and bk 64:92 at tile 25 (both hidden under
    the main loop); only the last 4 (b,k) rows ride the critical tail via
    3 direct SBUF->SBUF run copies into a 64-base staging tile (legal
    matmul base).  exp on ACT; per-(b,n) sums and the 1/sum broadcast are
    tiny f32r one-hot matmuls; final scale on DVE; two parallel DMAs
    write the [8,12,36,1] output slice.

All heavy inputs are host-repacked "partition-major" so every big DMA is
128 contiguous runs (one per partition).  Uploads are split across the
sync- and scalar-engine HWDGE rings in consumption order; the aggregate
~400GB/s HBM bandwidth is the startup binding constraint.
"""

import ml_dtypes
import numpy as np

import concourse.bacc as bacc
import concourse.mybir as mybir
import concourse.tile as tile
from concourse.bass_utils import run_bass_kernel_spmd

F32 = mybir.dt.float32
F32R = mybir.dt.float32r
NCORES = 8
B, K, N = 64, 12, 36
VD, QD, HID = 2048, 1024, 1024
BL = B // NCORES              # local batches per core
R = BL * K * N                # 3456 rows per core
BK = BL * K                   # 96 (b,k) groups per core
CC = VD // 128                # 16 contraction chunks over v-dim
QC = QD // 128                # 8 contraction chunks over q-dim
RC = 384                      # rows per DMA chunk (9 chunks)
NCH = R // RC
RT = 128                      # rows per PSUM tile
NT = RC // RT
NRT = R // RT                 # 27 row tiles
VSPLIT = 16                   # v-chunk DMA granularity (cc chunks per DMA)
SCALE = 1024.0                # host-side W1 scale so fp8 sees O(1) weights

_NC_CACHE = None

F8 = mybir.dt.float8e4
BF16 = mybir.dt.bfloat16
DR = mybir.MatmulPerfMode.DoubleRow


def _build_nc():
    nc = bacc.Bacc("TRN2", target_bir_lowering=False, debug=False,
                   num_devices=NCORES)

    def mm(out, lhsT, rhs, **kw):
        nc.tensor.matmul(out, lhsT, rhs, **kw)

    w1vt = nc.dram_tensor("w1vt", [128, CC, HID], F8, kind="ExternalInput").ap()
    # qt and W1q^T packed along the free dim: [:, cq, 0:96]=q^T, [:, cq, 96:1120]=W1q^T
    qtwq = nc.dram_tensor("qtwq", [128, QC, BK + HID], F8, kind="ExternalInput").ap()
    # bf16 constants: [:, 0:1024]=w2/S bcast, [0:96, 1024:2048]=b1*S replicated
    packb = nc.dram_tensor("packb", [128, HID + HID], BF16, kind="ExternalInput").ap()
    # f32r softmax selectors: [0:96, 0:8]=selb, [0:8, 8:104]=selb^T,
    # [64:68, 104:112]=selb rows 92:96 (the base-64 remainder copy),
    # [0:8, 112:180]=selb^T cols 92:96 remapped to out rows 64:68
    packf = nc.dram_tensor("packf", [128, BL + BK + BL + 68], F32R, kind="ExternalInput").ap()
    # v is split: the first two chunks ride with the weights at the front of
    # the upload order; the bulk uploads last, hidden under early compute.
    vth = nc.dram_tensor("vth", [2, 128, CC, RC], F8, kind="ExternalInput").ap()
    vtr = nc.dram_tensor("vtr", [NCH - 2, 128, CC, RC], F8, kind="ExternalInput").ap()
    out = nc.dram_tensor("out", [BL, K, N, 1], F32, kind="ExternalOutput").ap()

    MAX = mybir.AluOpType.max
    MULT = mybir.AluOpType.mult
    BYPASS = mybir.AluOpType.bypass
    ADD = mybir.AluOpType.add

    with tile.TileContext(nc) as tc:
        with tc.tile_pool(name="const", bufs=1) as cpool, \
             tc.tile_pool(name="wv", bufs=1) as wvpool, \
             tc.tile_pool(name="vtp", bufs=2) as vtpool, \
             tc.tile_pool(name="work", bufs=3) as work, \
             tc.tile_pool(name="small", bufs=1) as small, \
             tc.tile_pool(name="dram", bufs=1, space="DRAM") as dpool, \
             tc.tile_pool(name="psum", bufs=4, space="PSUM") as pspool:

            # ---- startup uploads, issue order == consumption order.
            qt4 = []
            for j in range(QC // 2):
                t = cpool.tile([128, 2, BK + HID], F8, name=f"qt{j}")
                nc.sync.dma_start(out=t, in_=qtwq[:, 2 * j:2 * j + 2, :])
                qt4.append(t)

            def vt_chunk_tiles(ch):
                src_ap = vth[ch] if ch < 2 else vtr[ch - 2]
                tiles = []
                eng = nc.scalar if ch == 0 else nc.sync
                for j in range(CC // VSPLIT):
                    t = vtpool.tile([128, VSPLIT, RC], F8, tag=f"vt{j}")
                    eng.dma_start(
                        out=t, in_=src_ap[:, j * VSPLIT:(j + 1) * VSPLIT, :])
                    tiles.append(t)
                return tiles

            vt_cur = vt_chunk_tiles(0)

            packb_s = cpool.tile([128, HID + HID], BF16)
            w2b_s = packb_s[:, 0:HID]
            b1b_s = packb_s[0:BK, HID:HID + HID]
            # b1 half first: it gates the hq eviction
            nc.sync.dma_start(out=packb_s[:, HID:HID + HID],
                              in_=packb[:, HID:HID + HID])
            nc.sync.dma_start(out=packb_s[:, 0:HID], in_=packb[:, 0:HID])

            WG = 4  # wv group size (cc chunks per DMA)
            wv_g = []
            for j in range(CC // WG):
                t = wvpool.tile([128, WG, HID], F8, tag=f"wvg{j}")
                eng = nc.scalar if j < 3 else nc.sync
                eng.dma_start(out=t, in_=w1vt[:, j * WG:(j + 1) * WG, :])
                wv_g.append(t)

            packf_s = cpool.tile([128, BL + BK + BL + 68], F32R)
            nc.sync.dma_start(out=packf_s, in_=packf)

            vt_next = vt_chunk_tiles(1)

            selb_s = packf_s[0:BK, 0:BL]
            selbt_s = packf_s[0:BL, BL:BL + BK]

            # per-row logits, laid out [p, rt] with row = rt*128 + p, split
            # into two tiles so the first half's softmax hides under the
            # main loop. 32 columns (StreamTranspose needs 32x32 blocks).
            NRT_A = 18   # 18*128 rows = 64 (b,k) groups — a 32-aligned bk split
            ls_a = cpool.tile([128, 32], F32)
            nc.vector.memset(ls_a, 0.0)
            ls_b = cpool.tile([128, 32], F32)
            nc.vector.memset(ls_b, 0.0)

            lg = dpool.tile([R], F32)
            lg2 = lg.rearrange("(t p) -> t p", t=NRT, p=128)

            hq_s = cpool.tile([BK, HID], BF16)
            s96 = small.tile([BK, N], F32)
            e96 = small.tile([BK, N], F32R)
            s96c = small.tile([68, N], F32, name="s96c")
            e96c = small.tile([68, N], F32R, name="e96c")
            sums_ps = pspool.tile([BL, N], F32, tag="sm", bufs=2)
            lst2a = cpool.tile([32, 128], F32, name="lst2a")
            lst2b = cpool.tile([32, 128], F32, name="lst2b")
            lgbk = lg.rearrange("(bk n) -> bk n", n=N)

            def build_lst2(ls, ls_t2):
                # ls[p, c] -> ls_t2[c, p] via 4 32x32 block transposes
                for i in range(4):
                    nc.vector.transpose(
                        ls_t2[0:32, 32 * i:32 * i + 32],
                        ls[32 * i:32 * i + 32, 0:32])

            PV = VSPLIT // 2   # DoubleRow cc-pairs per v DMA tile
            PW = WG // 2       # DoubleRow cc-pairs per wv group

            def emit_vmms(t, ps):
                for cp in range(CC // 2):
                    lhsT = vt_cur[cp // PV][:, (cp % PV) * 2:(cp % PV) * 2 + 2,
                                            t * RT:(t + 1) * RT]
                    wvc = wv_g[cp // PW][:, (cp % PW) * 2:(cp % PW) * 2 + 2, :]
                    mm(ps[:, 0:512], lhsT, wvc[:, :, 0:512],
                       start=(cp == 0), stop=(cp == CC // 2 - 1), perf_mode=DR)
                    mm(ps[:, 512:1024], lhsT, wvc[:, :, 512:1024],
                       start=(cp == 0), stop=(cp == CC // 2 - 1), perf_mode=DR)

            def emit_gather(rt):
                # broadcast-DMA the per-row hq vectors for this tile's rows
                # out of DRAM: one small DMA per (b,k) group intersecting the
                # tile, each replicating a single 4KB row across partitions.
                hqr = work.tile([128, HID], BF16, tag="hqr", bufs=3)
                g0 = (rt * RT) // N
                g1 = (rt * RT + RT - 1) // N
                eng = nc.sync if rt % 2 == 0 else nc.scalar
                for g in range(g0, g1 + 1):
                    a = max(0, N * g - rt * RT)
                    b = min(RT, N * (g + 1) - rt * RT)
                    eng.dma_start(
                        out=hqr[a:b, :],
                        in_=hq_dram[g:g + 1, :].to_broadcast((b - a, HID)))
                return hqr

            def emit_closer(rt, ps, hqr):
                joint = work.tile([128, HID], BF16, tag="joint")
                nc.vector.scalar_tensor_tensor(
                    out=joint, in0=ps, scalar=0.0, in1=hqr,
                    op0=BYPASS, op1=ADD)
                relu_w2 = work.tile([128, HID], BF16, tag="relu_w2")
                ls, col = (ls_a, rt) if rt < NRT_A else (ls_b, rt - NRT_A)
                nc.vector.scalar_tensor_tensor(
                    out=relu_w2, in0=joint, scalar=0.0, in1=w2b_s,
                    op0=MAX, op1=MULT,
                    accum_out=ls[:, col:col + 1])
                if rt == NRT_A - 1:
                    # flush + start the softmax head for bk rows 0:64 while
                    # the main loop still runs
                    build_lst2(ls_a, lst2a)
                    nc.sync.dma_start(out=lg2[0:NRT_A, :],
                                      in_=lst2a[0:NRT_A, :])
                    nc.sync.dma_start(out=s96[0:64, :], in_=lgbk[0:64, :])
                    nc.scalar.activation(e96[0:64, :], s96[0:64, :],
                                         mybir.ActivationFunctionType.Exp)
                    mm(sums_ps, selb_s[0:64, :], e96[0:64, :],
                       start=True, stop=False)
                if rt == NRT - 2:
                    # stage bk 64:92 (fully contained in tiles 18..25) so
                    # only the 4-row remainder rides the critical tail
                    build_lst2(ls_b, lst2b)
                    nc.sync.dma_start(out=lg2[NRT_A:NRT - 1, :],
                                      in_=lst2b[0:NRT - 1 - NRT_A, :])
                    nc.sync.dma_start(out=s96[64:92, :], in_=lgbk[64:92, :])
                    nc.scalar.activation(e96[64:92, :], s96[64:92, :],
                                         mybir.ActivationFunctionType.Exp)
                    mm(sums_ps, selb_s[64:92, :], e96[64:92, :],
                       start=False, stop=False)

            # ---- PE warmup: ~3.6us of junk DoubleRow matmuls with no DMA
            # dependencies, so the HAM clock-gate reaches 8/8 while the
            # startup uploads are still streaming.
            junk = work.tile([128, 1280], F8, tag="junk", bufs=1)
            nc.vector.memset(junk, 0.0)
            junk_ps = pspool.tile([128, 512], F32, tag="sm", bufs=2,
                                  name="junk_ps")
            for _ in range(14):
                mm(junk_ps, junk.rearrange("p (two m) -> p two m", two=2)[:, :, 0:128],
                   junk.rearrange("p (two m) -> p two m", two=2)[:, :, 128:640],
                   start=True, stop=True, perf_mode=DR)

            # ---- hq[bk, h] = q @ W1q^T (+ b1*S on eviction), via fp8
            # DoubleRow; first in PE program order so the PE starts as soon
            # as qtwq (the first upload) lands.
            hq_ps = [pspool.tile([BK, 512], F32, tag="sm", bufs=2,
                                 name=f"hq_ps{i}") for i in range(2)]
            for cp in range(QC // 2):
                src_t = qt4[cp]
                for half in range(2):
                    mm(hq_ps[half],
                       src_t[:, :, 0:BK],
                       src_t[:, :, BK + half * 512:BK + (half + 1) * 512],
                       start=(cp == 0), stop=(cp == QC // 2 - 1),
                       perf_mode=DR)
            for half in range(2):
                hs = slice(half * 512, (half + 1) * 512)
                # psum -> SBUF with the b1*S row added
                nc.vector.scalar_tensor_tensor(
                    out=hq_s[:, hs], in0=hq_ps[half], scalar=0.0,
                    in1=b1b_s[:, hs],
                    op0=BYPASS, op1=ADD)
            hq_dram = dpool.tile([BK, HID], BF16)
            nc.scalar.dma_start(out=hq_dram, in_=hq_s)

            # short filler so the PE never idles across a HAM window while
            # the first W1v group lands
            junk_ps2 = pspool.tile([128, 512], F32, tag="sm", bufs=2,
                                   name="junk_ps2")
            for _ in range(6):
                mm(junk_ps2,
                   junk.rearrange("p (two m) -> p two m", two=2)[:, :, 0:128],
                   junk.rearrange("p (two m) -> p two m", two=2)[:, :, 128:640],
                   start=True, stop=True, perf_mode=DR)

            # ---- chunk 0: v-matmuls for tiles 0..2 first; the closers are
            # deferred until the hq/oneh constants are certainly ready.
            ps0 = [pspool.tile([128, HID], F32, tag="ps", bufs=3,
                               name=f"ps0_{t}") for t in range(NT)]
            hqr0 = [emit_gather(t) for t in range(NT)]
            # cp-major: consume each wv group across all 3 tiles as soon as
            # its upload lands, so the PE never waits for the full W1v set.
            for cp in range(CC // 2 - 2):
                wvc = wv_g[cp // PW][:, (cp % PW) * 2:(cp % PW) * 2 + 2, :]
                for t in range(NT):
                    lhsT = vt_cur[cp // PV][:, (cp % PV) * 2:(cp % PV) * 2 + 2,
                                            t * RT:(t + 1) * RT]
                    mm(ps0[t][:, 0:512], lhsT, wvc[:, :, 0:512],
                       start=(cp == 0), stop=False, perf_mode=DR)
                    mm(ps0[t][:, 512:1024], lhsT, wvc[:, :, 512:1024],
                       start=(cp == 0), stop=False, perf_mode=DR)
            # tile-major tails: each tile's eviction starts while the next
            # tile's last matmuls still stream
            for t in range(NT):
                for cp in (CC // 2 - 2, CC // 2 - 1):
                    wvc = wv_g[cp // PW][:, (cp % PW) * 2:(cp % PW) * 2 + 2, :]
                    lhsT = vt_cur[cp // PV][:, (cp % PV) * 2:(cp % PV) * 2 + 2,
                                            t * RT:(t + 1) * RT]
                    mm(ps0[t][:, 0:512], lhsT, wvc[:, :, 0:512],
                       start=False, stop=(cp == CC // 2 - 1), perf_mode=DR)
                    mm(ps0[t][:, 512:1024], lhsT, wvc[:, :, 512:1024],
                       start=False, stop=(cp == CC // 2 - 1), perf_mode=DR)
                emit_closer(t, ps0[t], hqr0[t])
            vt_cur = vt_next

            # ---- chunks 1..8
            for ch in range(1, NCH):
                for t in range(NT):
                    rt = ch * NT + t
                    hqr = emit_gather(rt)
                    if t == 0 and ch + 1 < NCH:
                        vt_next = vt_chunk_tiles(ch + 1)
                    ps = pspool.tile([128, HID], F32, tag="ps", bufs=3)
                    if rt == NRT - 1:
                        # half-split: evict half 0 while half 1 still streams
                        lsh = work.tile([128, 2], F32, tag="acc2")
                        jointx = work.tile([128, HID], BF16, tag="joint")
                        rwx = work.tile([128, HID], BF16, tag="relu_w2")
                        for half in range(2):
                            hs = slice(half * 512, (half + 1) * 512)
                            for cp in range(CC // 2):
                                lhsT = vt_cur[cp // PV][
                                    :, (cp % PV) * 2:(cp % PV) * 2 + 2,
                                    t * RT:(t + 1) * RT]
                                wvc = wv_g[cp // PW][
                                    :, (cp % PW) * 2:(cp % PW) * 2 + 2, :]
                                mm(ps[:, hs], lhsT, wvc[:, :, hs],
                                   start=(cp == 0),
                                   stop=(cp == CC // 2 - 1), perf_mode=DR)
                            nc.vector.scalar_tensor_tensor(
                                out=jointx[:, hs], in0=ps[:, hs], scalar=0.0,
                                in1=hqr[:, hs], op0=BYPASS, op1=ADD)
                            nc.vector.scalar_tensor_tensor(
                                out=rwx[:, hs], in0=jointx[:, hs], scalar=0.0,
                                in1=w2b_s[:, hs], op0=MAX, op1=MULT,
                                accum_out=lsh[:, half:half + 1])
                        nc.vector.scalar_tensor_tensor(
                            out=ls_b[:, NRT - 1 - NRT_A:NRT - NRT_A],
                            in0=lsh[:, 0:1], scalar=0.0, in1=lsh[:, 1:2],
                            op0=BYPASS, op1=ADD)
                    else:
                        emit_vmms(t, ps)
                        emit_closer(rt, ps, hqr)
                vt_cur = vt_next

            # ---- flush the last tile's logits, finish the softmax.
            # The 4 remaining (b,k) rows (bk 92:96 = r 3312:3456) live in
            # lst2b partitions 7:9; copy their runs straight SBUF->SBUF
            # (1-descriptor DMAs), once into s96[92:96] for the final scale
            # and once into s96c[64:68] (a legal 64-base copy for the
            # partial-sums matmul).
            build_lst2(ls_b, lst2b)
            nc.sync.dma_start(out=s96c[64:65, 0:16],
                              in_=lst2b[7:8, 112:128])
            nc.scalar.dma_start(out=s96c[64:65, 16:36],
                                in_=lst2b[8:9, 0:20])
            nc.sync.dma_start(out=s96c[65:68, :], in_=lst2b[8:9, 20:128])
            nc.scalar.activation(e96c[64:68, :], s96c[64:68, :],
                                 mybir.ActivationFunctionType.Exp)
            mm(sums_ps, packf_s[64:68, BL + BK:BL + BK + BL], e96c[64:68, :],
               start=False, stop=True)
            rcp = small.tile([BL, N], F32R)
            with nc.allow_low_precision(reason="f32r is full fp32 bits"):
                nc.vector.reciprocal(rcp, sums_ps)
            rexp_ps = pspool.tile([BK, N], F32, tag="sm", bufs=2)
            mm(rexp_ps, selbt_s, rcp, start=True, stop=True)
            # the 4 remainder rows ride a 64-base copy: col-tiled broadcast
            # matmul + their own scale/DMA
            rexpc_ps = pspool.tile([68, N], F32, tag="sm", bufs=2,
                                   name="rexpc_ps")
            mm(rexpc_ps, packf_s[0:BL, BL + BK + BL:BL + BK + BL + 68], rcp,
               start=True, stop=True)
            w96 = small.tile([BK, N], F32)
            nc.vector.scalar_tensor_tensor(
                out=w96[0:92, :], in0=e96[0:92, :], scalar=0.0,
                in1=rexp_ps[0:92, :], op0=BYPASS, op1=MULT)
            w96x = small.tile([68, N], F32, name="w96x")
            nc.vector.scalar_tensor_tensor(
                out=w96x[64:68, :], in0=e96c[64:68, :], scalar=0.0,
                in1=rexpc_ps[64:68, :], op0=BYPASS, op1=MULT)
            outbk = out.rearrange("b k n o -> (b k) (n o)")
            nc.sync.dma_start(out=outbk[0:92, :], in_=w96[0:92, :])
            nc.scalar.dma_start(out=outbk[92:96, :], in_=w96x[64:68, :])

    nc.compile()
    return nc


def _get_nc():
    global _NC_CACHE
    if _NC_CACHE is None:
        _NC_CACHE = _build_nc()
    return _NC_CACHE


def _prepare_in_maps(inputs):
    v = np.asarray(inputs["v"], dtype=np.float32)
    q = np.asarray(inputs["q"], dtype=np.float32)
    W1 = np.asarray(inputs["W1"], dtype=np.float32)
    g1 = np.float64(np.asarray(inputs["g1"]))
    b1 = np.asarray(inputs["b1"], dtype=np.float32)
    W2 = np.asarray(inputs["W2"], dtype=np.float32)
    g2 = np.float64(np.asarray(inputs["g2"]))
    # b2 is a scalar added to every logit -> cancels in softmax over k.

    W1e = ((g1 / np.linalg.norm(W1.astype(np.float64))) * W1).astype(np.float32)
    W2e = ((g2 / np.linalg.norm(W2.astype(np.float64))) * W2).astype(np.float32)

    BF = ml_dtypes.bfloat16
    FP8 = ml_dtypes.float8_e4m3
    W1s = (W1e * SCALE).astype(np.float32)
    # partition-major repacks: [..., 128 p, chunk, inner]
    w1vt = np.ascontiguousarray(                       # [128, 16, 1024]
        W1s[:, :VD].T.reshape(CC, 128, HID).transpose(1, 0, 2)).astype(FP8)
    w1qt = W1s[:, VD:].T.reshape(QC, 128, HID).transpose(1, 0, 2)  # [128, 8, 1024]
    selb = (np.arange(BL)[None, :] == (np.arange(BK) // K)[:, None]).astype(np.float32)


    packb = np.zeros((128, HID + HID), dtype=BF)
    packb[:, 0:HID] = (W2e.reshape(1, HID) / SCALE).astype(BF)
    packb[0:BK, HID:HID + HID] = (b1.reshape(1, HID) * SCALE).astype(BF)
    packf = np.zeros((128, BL + BK + BL + 68), dtype=np.float32)
    packf[0:BK, 0:BL] = selb
    packf[0:BL, BL:BL + BK] = selb.T
    packf[64:68, BL + BK:BL + BK + BL] = selb[92:96]
    packf[0:BL, BL + BK + BL + 64:BL + BK + BL + 68] = selb.T[:, 92:96]

    shared = dict(w1vt=w1vt, packb=packb, packf=packf)
    in_maps = []
    for c in range(NCORES):
        vl = v[c * BL:(c + 1) * BL].reshape(R, VD)
        # vt[ch, p, cc, r_in_chunk] = v[ch*RC + r, cc*128 + p]
        vt4 = np.ascontiguousarray(
            vl.T.reshape(CC, 128, NCH, RC).transpose(2, 1, 0, 3)).astype(FP8)
        ql = q[c * BL:(c + 1) * BL].reshape(BK, QD)
        qt3 = ql.T.reshape(QC, 128, BK).transpose(1, 0, 2)   # [128, 8, 96]
        qtwq = np.concatenate([qt3, w1qt], axis=2).astype(FP8)  # [128, 8, 1120]
        in_maps.append(dict(vth=np.ascontiguousarray(vt4[:2]),
                            vtr=np.ascontiguousarray(vt4[2:]),
                            qtwq=np.ascontiguousarray(qtwq), **shared))
    return in_maps


def kernel(**inputs) -> np.ndarray:
    in_maps = _prepare_in_maps(inputs)
    nc = _get_nc()
    res = run_bass_kernel_spmd(nc, in_maps, list(range(NCORES)))
    outs = [res.results[c]["out"].reshape(BL, K, N, 1) for c in range(NCORES)]
    return np.concatenate(outs, axis=0)
